# revision 35
# baseline (speedup 1.0000x reference)
"""GQA kernel for Trainium2, 8 NeuronCores.

Sharding: core c = (b, g) with b = c // 4 (batch), g = c % 4 (KV group).
Each core computes, for its batch b and group g (4 query heads, 1 KV head):
  qT[d, t] for the 4 heads, kT[d, t], v[t, d] projections (contraction over EMB,
  inputs pre-packed on host so EMB lands on SBUF partitions),
  causal flash-style attention in [k-part, q-free] score layout,
  and the partial output projection  partial_g = (attn out) @ Wp[:, g cols].T.
Host gathers: y[b] = sum_g upcast(partial[b, g]) + bp (partials stored bf16
to halve the output DMA).

All matmuls run in bf16 (fp32 PSUM accumulation); host pre-casts inputs.
Inputs are host-packed to the exact SBUF layout [128, free] so each tensor
loads with one contiguous DMA, interleaved across the two HWDGE queues
(sync/scalar) so descriptor feed ramps fast.
Causal structure: scores for the diagonal 128-row k-blocks are trimmed to the
q-columns that can attend; only the first 128-col band of each diagonal block
needs an elementwise triangular mask.

Scheduling notes (each validated against NTFF traces):
- PSUM is carved as two 2-bank [128,1024] "mm" slots + 2 oext + 2 tr banks.
  Projection chains pack pairwise into the mm slots; score blocks pack
  two-per-slot so ONE exp covers both (the per-op ACT overhead otherwise
  saturates the scalar engine during attention).
- The qp=0 attention iterations are interleaved into the Q-projection head
  loop, hiding attention's shallow-pipeline start inside dense Q matmuls.
- The output-projection epilogue rotates over 4 psum banks (oext+tr) with
  staging copies alternating DVE/ACT and DMA issues on the sync queue, so
  its matmul chains never wait on bank evacuation.
"""

import numpy as np
import ml_dtypes

T = 2048
EMB = 2048
HD = 128
GS = 4          # query heads per core (per KV group)
NE = EMB // 128 # 16 contraction chunks
NT = T // 128   # 16 row tiles
NQP = T // 512  # 4 q passes of 512
SCALE = float(HD) ** -0.5

_BF16 = ml_dtypes.bfloat16
_PROGRAM = None


def _build_program():
    import concourse.bass as bass
    import concourse.tile as tile
    from concourse import bacc, mybir
    from concourse.masks import make_identity

    f32 = mybir.dt.float32
    bf16 = mybir.dt.bfloat16

    nc = bacc.Bacc("TRN2", target_bir_lowering=False, debug=False)

    # all inputs host-packed to [128 partitions, free] SBUF layout
    xT_d = nc.dram_tensor("xTp", [128, NE * T], bf16, kind="ExternalInput")
    wq_d = nc.dram_tensor("wqp", [128, NE * GS * HD], bf16, kind="ExternalInput")
    wk_d = nc.dram_tensor("wkp", [128, NE * HD], bf16, kind="ExternalInput")
    wv_d = nc.dram_tensor("wvp", [128, NE * HD], bf16, kind="ExternalInput")
    wp_d = nc.dram_tensor("wpp", [128, GS * EMB], bf16, kind="ExternalInput")
    out_d = nc.dram_tensor("partial", [T, EMB], bf16, kind="ExternalOutput").rearrange(
        "(n p) m -> n p m", p=128
    )

    with tile.TileContext(nc) as tc:
        with (
            tc.tile_pool(name="big", bufs=1) as big,
            tc.tile_pool(name="pt", bufs=24) as ptp,
            tc.tile_pool(name="onorm", bufs=12) as onp,
            tc.tile_pool(name="ostage", bufs=4) as osp,
            tc.tile_pool(name="small", bufs=6) as smp,
            tc.tile_pool(name="mm", bufs=2, space="PSUM") as pmm,
            tc.tile_pool(name="oext", bufs=2, space="PSUM") as pox,
            tc.tile_pool(name="tr", bufs=2, space="PSUM") as ptr,
        ):
            xT_sb = big.tile([128, NE * T], bf16)
            wq_sb = big.tile([128, NE * GS * HD], bf16)
            wk_sb = big.tile([128, NE * HD], bf16)
            wv_sb = big.tile([128, NE * HD], bf16)
            wp_sb = big.tile([128, GS * EMB], bf16)
            qT_sb = big.tile([128, GS * T], bf16)
            kT_sb = big.tile([128, T], bf16)
            vT_sb = big.tile([128, T], bf16)
            vext_sb = big.tile([128, NT * (HD + 1)], bf16)
            ohT_sb = big.tile([128, GS * T], bf16)
            ident = big.tile([128, 128], bf16)
            mask = big.tile([128, 128], bf16)

            # constants: identity for PE transpose; triangular mask for the
            # first 128-col band of diagonal blocks (keep iff q_local >= k_local)
            make_identity(nc, ident)
            nc.gpsimd.memset(mask, 1.0)
            nc.gpsimd.affine_select(
                out=mask,
                in_=mask,
                compare_op=mybir.AluOpType.is_ge,
                fill=0.0,
                base=0,
                pattern=[[1, 128]],
                channel_multiplier=-1,
            )
            nc.vector.memset(vext_sb, 1.0)

            # input DMAs on the two HWDGE queues (sync + scalar — the only
            # hardware-descriptor queues): x chunks alternate between them so
            # descriptor feed ramps 2x faster; wk/wv lead the scalar queue
            # (KV chunk-0 needs them), wq/wp trail the x stream
            for c in range(0, NE, 2):
                nc.sync.dma_start(
                    out=xT_sb[:, c * T : (c + 1) * T],
                    in_=xT_d[:, c * T : (c + 1) * T],
                )
            nc.scalar.dma_start(out=wk_sb, in_=wk_d[:, :])
            nc.scalar.dma_start(out=wv_sb, in_=wv_d[:, :])
            for c in range(1, NE, 2):
                nc.scalar.dma_start(
                    out=xT_sb[:, c * T : (c + 1) * T],
                    in_=xT_d[:, c * T : (c + 1) * T],
                )
            hw = NE * GS * HD // 2
            for q in range(2):
                nc.sync.dma_start(
                    out=wq_sb[:, q * hw : (q + 1) * hw],
                    in_=wq_d[:, q * hw : (q + 1) * hw],
                )
            nc.sync.dma_start(out=wp_sb, in_=wp_d[:, :])

            # kT + vT projections interleaved, chunk-outer so PE consumes each
            # xT chunk as it arrives. kT chains pack pairwise into the two
            # 2-bank "mm" slots; vT borrows the attention pools' slots
            # (oext x2 + tr x2) so both run during the DMA-arrival window.
            kssA = pmm.tile([128, 1024], f32, tag="mm", name="kssA")
            kssB = pmm.tile([128, 1024], f32, tag="mm", name="kssB")
            # PE clock warmup: garbage matmuls into kssA (overwritten by the
            # real chain's start=True) with no upstream deps, so HAM reaches
            # 2.4 GHz while the first DMA chunks are still in flight.
            # ohT_sb is written only much later in the program (WAR only).
            for _ in range(28):
                nc.tensor.matmul(
                    kssA[:, 0:512], lhsT=ohT_sb[:, 0:128], rhs=ohT_sb[:, 0:512],
                    start=True, stop=True,
                )
            kss = [kssA[:, 0:512], kssA[:, 512:1024], kssB[:, 0:512], kssB[:, 512:1024]]
            vss = [
        pox.tile([128, 512], f32, tag="oext", name="vss0"),
        pox.tile([128, 512], f32, tag="oext", name="vss1"),
        ptr.tile([128, 512], f32, tag="tr", name="vss2"),
        ptr.tile([128, 512], f32, tag="tr", name="vss3"),
            ]
            for c in range(NE):
                for tp in range(4):
                    nc.tensor.matmul(
                        kss[tp],
                        lhsT=wk_sb[:, c * HD : (c + 1) * HD],
                        rhs=xT_sb[:, c * T + tp * 512 : c * T + (tp + 1) * 512],
                        start=(c == 0),
                        stop=(c == NE - 1),
                    )
                for tp in range(4):
                    nc.tensor.matmul(
                        vss[tp],
                        lhsT=wv_sb[:, c * HD : (c + 1) * HD],
                        rhs=xT_sb[:, c * T + tp * 512 : c * T + (tp + 1) * 512],
                        start=(c == 0),
                        stop=(c == NE - 1),
                    )
            # kss drains first (Q's first chains need its slot), on both
            # engines concurrently; vss after (v transposes run much later)
            nc.vector.tensor_copy(kT_sb[:, 0:1024], kssA)
            nc.scalar.copy(kT_sb[:, 1024:2048], kssB)
            for tp in range(4):
                eng = nc.vector if tp % 2 == 0 else nc.scalar
                if eng is nc.vector:
                    eng.tensor_copy(vT_sb[:, tp * 512 : (tp + 1) * 512], vss[tp])
                else:
                    eng.copy(vT_sb[:, tp * 512 : (tp + 1) * 512], vss[tp])
            for tt in range(NT):
                tv = ptr.tile([128, 128], bf16, tag="tr")
                nc.tensor.transpose(tv, vT_sb[:, tt * 128 : (tt + 1) * 128], ident)
                nc.vector.tensor_copy(
                    vext_sb[:, tt * (HD + 1) : tt * (HD + 1) + HD], tv
                )

            # qT projection happens below, interleaved with the qp=0
            # attention iterations (see the window loop)

            # attention + output projection, software-pipelined: scores for
            # iteration i+1 are emitted before AV of iteration i so the PE
            # stream never waits for ACT's exp backlog at AV chain heads
            deferred = []

            def emit_scores(qp, s):
                if qp == 0:
                    # runs inside the Q-projection phase: keep the "mm" slots
                    # free for the Q chains — the 4 diagonal blocks go
                    # unpaired into single-bank oext/tr tiles (ACT is idle
                    # here, so 4 small exps cost nothing) so a Q chain head
                    # never waits on a scores exp drain
                    pts = []
                    for o in range(4):
                        trim = 128 * o
                        w = 512 - trim
                        q0 = s * T + trim
                        pool, tag = (pox, "oext") if o % 2 == 0 else (ptr, "tr")
                        ps = pool.tile([128, 512], f32, tag=tag, name="ps0")
                        pt = ptp.tile([128, 1024], bf16, tag="pt", name="pt")
                        nc.tensor.matmul(
                            ps[:, 0:w],
                            lhsT=kT_sb[:, o * 128 : (o + 1) * 128],
                            rhs=qT_sb[:, q0 : q0 + w],
                            start=True,
                            stop=True,
                        )
                        nc.scalar.activation(
                            pt[:, 0:w], ps[:, 0:w],
                            mybir.ActivationFunctionType.Exp, scale=SCALE,
                        )
                        nc.vector.tensor_mul(
                            pt[:, 0:128], pt[:, 0:128], mask
                        )
                        pts.append((pt, trim, 0))
                    return pts
                # score blocks packed two-per-psum-pair-tile (each matmul
                # stays within one bank) so ONE exp covers two blocks —
                # halves the ACT per-op overhead, which otherwise saturates
                # the scalar engine during the attention phase.
                # groups: list of (j, trim, base) packed into one tile
                nfull = 4 * qp
                groups = [
                    [(2 * p, 0, 0), (2 * p + 1, 0, 512)]
                    for p in range(nfull // 2)
                ]
                # diagonal band: widths 512,384 share a tile; 256,128 share
                groups.append([(nfull, 0, 0), (nfull + 1, 128, 512)])
                groups.append([(nfull + 2, 256, 0), (nfull + 3, 384, 256)])
                pts = [None] * (nfull + 4)
                for grp in groups:
                    width = max(base + 512 - trim for (_, trim, base) in grp)
                    ps = pmm.tile([128, 1024], f32, tag="mm", name="ps")
                    pt = ptp.tile([128, 1024], bf16, tag="pt", name="pt")
                    for (j, trim, base) in grp:
                        w = 512 - trim
                        q0 = s * T + qp * 512 + trim
                        nc.tensor.matmul(
                            ps[:, base : base + w],
                            lhsT=kT_sb[:, j * 128 : (j + 1) * 128],
                            rhs=qT_sb[:, q0 : q0 + w],
                            start=True,
                            stop=True,
                        )
                    nc.scalar.activation(
                        pt[:, 0:width], ps[:, 0:width],
                        mybir.ActivationFunctionType.Exp, scale=SCALE,
                    )
                    for (j, trim, base) in grp:
                        if j - nfull >= 0:
                            # only the first 128-col band straddles the diagonal
                            nc.vector.tensor_mul(
                                pt[:, base : base + 128], pt[:, base : base + 128], mask
                            )
                        pts[j] = (pt, trim, base)
                return pts

            def emit_av(qp, s, pts):
                norms = []
                for u in range(4):
                    jmax = 4 * qp + u
                    # short chains early on: rotate over 4 banks (oext+tr) so
                    # the chain head never waits on DVE normalization drain
                    if qp < 2 and u % 2 == 1:
                        oe = ptr.tile([128, HD + 1], f32, tag="tr", name="oe")
                    else:
                        oe = pox.tile([128, HD + 1], f32, tag="oext", name="oe")
                    for j in range(jmax + 1):
                        pt, trim, base = pts[j]
                        c0 = base + u * 128 - trim
                        nc.tensor.matmul(
                            oe,
                            lhsT=pt[:, c0 : c0 + 128],
                            rhs=vext_sb[:, j * (HD + 1) : (j + 1) * (HD + 1)],
                            start=(j == 0),
                            stop=(j == jmax),
                        )
                    rc = smp.tile([128, 1], f32, tag="rc", name="rc")
                    nc.vector.reciprocal(rc, oe[:, HD : HD + 1])
                    on = onp.tile([128, 128], bf16, tag="on", name="on")
                    nc.vector.tensor_scalar_mul(on, oe[:, 0:HD], rc)
                    norms.append((on, s, qp * 512 + u * 128))
                return norms

            def emit_epilogue(qp):
                # output projection for this q-pass's 4 row tiles, staged DMA
                # per jp so the output drains early. PSUM rotates over FOUR
                # banks (oext x2 + tr x2) and the staging copies alternate
                # DVE/ACT so the matmul chains never wait on bank evacuation.
                for u in range(4):
                    tt = qp * 4 + u
                    for jp in range(4):
                        pool = pox if jp % 2 == 0 else ptr
                        tag = "oext" if jp % 2 == 0 else "tr"
                        ps = pool.tile([128, 512], f32, tag=tag, name="ps")
                        for s in range(GS):
                            nc.tensor.matmul(
                                ps,
                                lhsT=ohT_sb[:, s * T + tt * 128 : s * T + (tt + 1) * 128],
                                rhs=wp_sb[:, s * EMB + jp * 512 : s * EMB + (jp + 1) * 512],
                                start=(s == 0),
                                stop=(s == GS - 1),
                            )
                        ot = osp.tile([128, 512], bf16, tag="ostage", name="ot")
                        # staging on DVE only: epilogue(qp) executes during
                        # the qp+1 stretch where ACT runs near-saturated on
                        # exp — a 570ns copy in its strict FIFO stalls AV
                        # chain heads; DMA issues ride the idle sync queue
                        nc.vector.tensor_copy(ot, ps)
                        nc.sync.dma_start(
                            out=out_d[tt, :, jp * 512 : (jp + 1) * 512], in_=ot
                        )


            last_norms = []

            def emit_transposes(norms):
                for on, s, tq in norms:
                    tps = ptr.tile([128, 128], bf16, tag="tr", name="tps")
                    nc.tensor.transpose(tps, on, ident)
                    nc.vector.tensor_copy(
                        ohT_sb[:, s * T + tq : s * T + tq + 128], tps
                    )

            def advance(pending):
                # AV for the pending iteration, then the (lag-1) transposes of
                # the previous one; at a q-pass boundary flush and project
                nonlocal last_norms
                qp, s, pts = pending
                norms = emit_av(qp, s, pts)
                emit_transposes(last_norms)
                last_norms = norms
                if s == GS - 1:
                    emit_transposes(last_norms)
                    last_norms = []
                    emit_epilogue(qp)

            window = []

            def push(qp, s, maxw):
                pts = emit_scores(qp, s)
                window.append((qp, s, pts))
                if len(window) > maxw:
                    advance(window.pop(0))

            # Q projection per head, in half-head groups that ping-pong the
            # two 2-bank "mm" slots (copy of group k overlaps chains of group
            # k+1). The qp=0 attention iteration for head s rides along right
            # after its qT is staged, so attention's shallow-pipeline start
            # hides inside dense Q-projection matmul work.
            for s in range(GS):
                for h, tps in enumerate(((0, 1), (2, 3))):
                    pg = pmm.tile([128, 1024], f32, tag="mm", name="pg")
                    for c in range(NE):
                        for ti, tp in enumerate(tps):
                            nc.tensor.matmul(
                                pg[:, ti * 512 : (ti + 1) * 512],
                                lhsT=wq_sb[
                                    :, c * GS * HD + s * HD : c * GS * HD + (s + 1) * HD
                                ],
                                rhs=xT_sb[:, c * T + tp * 512 : c * T + (tp + 1) * 512],
                                start=(c == 0),
                                stop=(c == NE - 1),
                            )
                    dst = qT_sb[:, s * T + tps[0] * 512 : s * T + (tps[1] + 1) * 512]
                    if (2 * s + h) % 2 == 0:
                        nc.vector.tensor_copy(dst, pg)
                    else:
                        nc.scalar.copy(dst, pg)
                push(0, s, 3)
            for qp in range(1, NQP):
                for s in range(GS):
                    # deeper pipeline early on: qp=1 AV chains are short, so
                    # PE needs more queued scores to ride out ACT exp latency
                    push(qp, s, 3 if qp < 2 else 2)
            for w in window:
                advance(w)

    nc.finalize()
    return nc


def _get_program():
    global _PROGRAM
    if _PROGRAM is None:
        _PROGRAM = _build_program()
    return _PROGRAM


def _pack(a, nchunk):
    """[nchunk*128, F] -> [128, nchunk*F] so it lands in SBUF layout with one
    contiguous DMA: out[p, c*F + f] = a[c*128 + p, f]."""
    n, f = a.shape
    assert n == nchunk * 128
    return np.ascontiguousarray(
        a.reshape(nchunk, 128, f).transpose(1, 0, 2).reshape(128, nchunk * f)
    )


def _make_in_maps(x, Wq, Wk, Wv, Wp):
    # convert to numpy up front: slicing a jax array would trace/compile
    # a jax op per slice instead of cheap host-side numpy views
    x, Wq, Wk, Wv, Wp = (np.asarray(a) for a in (x, Wq, Wk, Wv, Wp))
    in_maps = []
    xTs = [_pack(x[b].T.astype(_BF16), NE) for b in range(2)]
    for c in range(8):
        b, g = c // 4, c % 4
        sl = slice(g * GS * HD, (g + 1) * GS * HD)
        kv = slice(g * GS * HD, g * GS * HD + HD)
        in_maps.append(
            {
                "xTp": xTs[b],
                "wqp": _pack(Wq[sl, :].T.astype(_BF16), NE),
                "wkp": _pack(Wk[kv, :].T.astype(_BF16), NE),
                "wvp": _pack(Wv[kv, :].T.astype(_BF16), NE),
                "wpp": _pack(Wp[:, sl].T.astype(_BF16), GS),
            }
        )
    return in_maps


def run(x, Wq, Wk, Wv, Wp, bp, trace=False, **trace_kwargs):
    from concourse.bass_utils import run_bass_kernel_spmd

    nc = _get_program()
    in_maps = _make_in_maps(x, Wq, Wk, Wv, Wp)
    res = run_bass_kernel_spmd(
        nc, in_maps, core_ids=list(range(8)), trace=trace, **trace_kwargs
    )
    bp = np.asarray(bp, dtype=np.float32)
    y = np.empty((2, T, EMB), dtype=np.float32)
    for b in range(2):
        acc = res.results[4 * b]["partial"].astype(np.float32)
        for g in range(1, 4):
            acc += res.results[4 * b + g]["partial"].astype(np.float32)
        y[b] = acc + bp
    return y, res


def kernel(x, Wq, Wk, Wv, Wp, bp):
    y, _ = run(x, Wq, Wk, Wv, Wp, bp, trace=False)
    return y



# revision 36
# speedup vs baseline: 1.1843x; 1.1843x over previous
"""GQA kernel for Trainium2, 8 NeuronCores.

Sharding: core c = (b, g) with b = c // 4 (batch), g = c % 4 (KV group).
Each core computes, for its batch b and group g (4 query heads, 1 KV head):
  qT[d, t] for the 4 heads, kT[d, t], v[t, d] projections (contraction over EMB,
  inputs pre-packed on host so EMB lands on SBUF partitions),
  causal flash-style attention in [k-part, q-free] score layout,
  and the partial output projection  partial_g = (attn out) @ Wp[:, g cols].T.
Host gathers: y[b] = sum_g upcast(partial[b, g]) + bp (partials stored bf16
to halve the output DMA).

All matmuls run in bf16 (fp32 PSUM accumulation); host pre-casts inputs.
Inputs are host-packed to the exact SBUF layout [128, free] so each tensor
loads with one contiguous DMA, interleaved across the two HWDGE queues
(sync/scalar) so descriptor feed ramps fast.
Causal structure: scores for the diagonal 128-row k-blocks are trimmed to the
q-columns that can attend; only the first 128-col band of each diagonal block
needs an elementwise triangular mask.

Scheduling notes (each validated against NTFF traces):
- PSUM is carved as two 2-bank [128,1024] "mm" slots + 2 oext + 2 tr banks.
  Projection chains pack pairwise into the mm slots; score blocks pack
  two-per-slot so ONE exp covers both (the per-op ACT overhead otherwise
  saturates the scalar engine during attention).
- The qp=0 attention iterations are interleaved into the Q-projection head
  loop, hiding attention's shallow-pipeline start inside dense Q matmuls.
- The output-projection epilogue rotates over 4 psum banks (oext+tr) with
  staging copies alternating DVE/ACT and DMA issues on the sync queue, so
  its matmul chains never wait on bank evacuation.
"""

import numpy as np
import ml_dtypes

T = 2048
EMB = 2048
HD = 128
GS = 4          # query heads per core (per KV group)
NE = EMB // 128 # 16 contraction chunks
NT = T // 128   # 16 row tiles
NQP = T // 512  # 4 q passes of 512
SCALE = float(HD) ** -0.5

_BF16 = ml_dtypes.bfloat16
_PROGRAM = None


def _build_program():
    import concourse.bass as bass
    import concourse.tile as tile
    from concourse import bacc, mybir
    from concourse.masks import make_identity

    f32 = mybir.dt.float32
    bf16 = mybir.dt.bfloat16

    nc = bacc.Bacc("TRN2", target_bir_lowering=False, debug=False)

    # all inputs host-packed to [128 partitions, free] SBUF layout
    xT_d = nc.dram_tensor("xTp", [128, NE * T], bf16, kind="ExternalInput")
    wq_d = nc.dram_tensor("wqp", [128, NE * GS * HD], bf16, kind="ExternalInput")
    wk_d = nc.dram_tensor("wkp", [128, NE * HD], bf16, kind="ExternalInput")
    wv_d = nc.dram_tensor("wvp", [128, NE * HD], bf16, kind="ExternalInput")
    wp_d = nc.dram_tensor("wpp", [128, GS * EMB], bf16, kind="ExternalInput")
    out_d = nc.dram_tensor("partial", [T, EMB], bf16, kind="ExternalOutput").rearrange(
        "(n p) m -> n p m", p=128
    )

    with tile.TileContext(nc) as tc:
        with (
            tc.tile_pool(name="big", bufs=1) as big,
            tc.tile_pool(name="pt", bufs=24) as ptp,
            tc.tile_pool(name="onorm", bufs=12) as onp,
            tc.tile_pool(name="ostage", bufs=4) as osp,
            tc.tile_pool(name="small", bufs=6) as smp,
            tc.tile_pool(name="mm", bufs=2, space="PSUM") as pmm,
            tc.tile_pool(name="oext", bufs=2, space="PSUM") as pox,
            tc.tile_pool(name="tr", bufs=2, space="PSUM") as ptr,
        ):
            xT_sb = big.tile([128, NE * T], bf16)
            wq_sb = big.tile([128, NE * GS * HD], bf16)
            wk_sb = big.tile([128, NE * HD], bf16)
            wv_sb = big.tile([128, NE * HD], bf16)
            wp_sb = big.tile([128, GS * EMB], bf16)
            qT_sb = big.tile([128, GS * T], bf16)
            kT_sb = big.tile([128, T], bf16)
            vT_sb = big.tile([128, T], bf16)
            vext_sb = big.tile([128, NT * (HD + 1)], bf16)
            ohT_sb = big.tile([128, GS * T], bf16)
            ident = big.tile([128, 128], bf16)
            mask = big.tile([128, 128], bf16)

            # constants: identity for PE transpose; triangular mask for the
            # first 128-col band of diagonal blocks (keep iff q_local >= k_local)
            make_identity(nc, ident)
            nc.gpsimd.memset(mask, 1.0)
            nc.gpsimd.affine_select(
                out=mask,
                in_=mask,
                compare_op=mybir.AluOpType.is_ge,
                fill=0.0,
                base=0,
                pattern=[[1, 128]],
                channel_multiplier=-1,
            )
            nc.vector.memset(vext_sb, 1.0)

            # input DMAs on the two HWDGE queues (sync + scalar — the only
            # hardware-descriptor queues): x chunks alternate between them so
            # descriptor feed ramps 2x faster; wk/wv lead the scalar queue
            # (KV chunk-0 needs them), wq/wp trail the x stream
            for c in range(0, NE, 2):
                nc.sync.dma_start(
                    out=xT_sb[:, c * T : (c + 1) * T],
                    in_=xT_d[:, c * T : (c + 1) * T],
                )
            nc.scalar.dma_start(out=wk_sb, in_=wk_d[:, :])
            nc.scalar.dma_start(out=wv_sb, in_=wv_d[:, :])
            for c in range(1, NE, 2):
                nc.scalar.dma_start(
                    out=xT_sb[:, c * T : (c + 1) * T],
                    in_=xT_d[:, c * T : (c + 1) * T],
                )
            hw = NE * GS * HD // 2
            for q in range(2):
                nc.sync.dma_start(
                    out=wq_sb[:, q * hw : (q + 1) * hw],
                    in_=wq_d[:, q * hw : (q + 1) * hw],
                )
            nc.sync.dma_start(out=wp_sb, in_=wp_d[:, :])

            # kT + vT projections interleaved, chunk-outer so PE consumes each
            # xT chunk as it arrives. kT chains pack pairwise into the two
            # 2-bank "mm" slots; vT borrows the attention pools' slots
            # (oext x2 + tr x2) so both run during the DMA-arrival window.
            kssA = pmm.tile([128, 1024], f32, tag="mm", name="kssA")
            kssB = pmm.tile([128, 1024], f32, tag="mm", name="kssB")
            # PE clock warmup: garbage matmuls into kssA (overwritten by the
            # real chain's start=True) with no upstream deps, so HAM reaches
            # 2.4 GHz while the first DMA chunks are still in flight.
            # ohT_sb is written only much later in the program (WAR only).
            for _ in range(28):
                nc.tensor.matmul(
                    kssA[:, 0:512], lhsT=ohT_sb[:, 0:128], rhs=ohT_sb[:, 0:512],
                    start=True, stop=True,
                )
            kss = [kssA[:, 0:512], kssA[:, 512:1024], kssB[:, 0:512], kssB[:, 512:1024]]
            vss = [
        pox.tile([128, 512], f32, tag="oext", name="vss0"),
        pox.tile([128, 512], f32, tag="oext", name="vss1"),
        ptr.tile([128, 512], f32, tag="tr", name="vss2"),
        ptr.tile([128, 512], f32, tag="tr", name="vss3"),
            ]
            for c in range(NE):
                for tp in range(4):
                    nc.tensor.matmul(
                        kss[tp],
                        lhsT=wk_sb[:, c * HD : (c + 1) * HD],
                        rhs=xT_sb[:, c * T + tp * 512 : c * T + (tp + 1) * 512],
                        start=(c == 0),
                        stop=(c == NE - 1),
                    )
                for tp in range(4):
                    nc.tensor.matmul(
                        vss[tp],
                        lhsT=wv_sb[:, c * HD : (c + 1) * HD],
                        rhs=xT_sb[:, c * T + tp * 512 : c * T + (tp + 1) * 512],
                        start=(c == 0),
                        stop=(c == NE - 1),
                    )
            # kss drains first (Q's first chains need its slot), on both
            # engines concurrently; vss after (v transposes run much later)
            nc.vector.tensor_copy(kT_sb[:, 0:1024], kssA)
            nc.scalar.copy(kT_sb[:, 1024:2048], kssB)
            for tp in range(4):
                eng = nc.vector if tp % 2 == 0 else nc.scalar
                if eng is nc.vector:
                    eng.tensor_copy(vT_sb[:, tp * 512 : (tp + 1) * 512], vss[tp])
                else:
                    eng.copy(vT_sb[:, tp * 512 : (tp + 1) * 512], vss[tp])
            for tt in range(NT):
                tv = ptr.tile([128, 128], bf16, tag="tr")
                nc.tensor.transpose(tv, vT_sb[:, tt * 128 : (tt + 1) * 128], ident)
                nc.vector.tensor_copy(
                    vext_sb[:, tt * (HD + 1) : tt * (HD + 1) + HD], tv
                )

            # qT projection happens below, interleaved with the qp=0
            # attention iterations (see the window loop)

            # attention + output projection, software-pipelined: scores for
            # iteration i+1 are emitted before AV of iteration i so the PE
            # stream never waits for ACT's exp backlog at AV chain heads
            deferred = []

            def emit_scores(qp, s):
                if qp == 0:
                    # runs inside the Q-projection phase: keep the "mm" slots
                    # free for the Q chains — the 4 diagonal blocks go
                    # unpaired into single-bank oext/tr tiles (ACT is idle
                    # here, so 4 small exps cost nothing) so a Q chain head
                    # never waits on a scores exp drain
                    pts = []
                    for o in range(4):
                        trim = 128 * o
                        w = 512 - trim
                        q0 = s * T + trim
                        pool, tag = (pox, "oext") if o % 2 == 0 else (ptr, "tr")
                        ps = pool.tile([128, 512], f32, tag=tag, name="ps0")
                        pt = ptp.tile([128, 1024], bf16, tag="pt", name="pt")
                        nc.tensor.matmul(
                            ps[:, 0:w],
                            lhsT=kT_sb[:, o * 128 : (o + 1) * 128],
                            rhs=qT_sb[:, q0 : q0 + w],
                            start=True,
                            stop=True,
                        )
                        nc.scalar.activation(
                            pt[:, 0:w], ps[:, 0:w],
                            mybir.ActivationFunctionType.Exp, scale=SCALE,
                        )
                        nc.vector.tensor_mul(
                            pt[:, 0:128], pt[:, 0:128], mask
                        )
                        pts.append((pt, trim, 0))
                    return pts
                # score blocks packed two-per-psum-pair-tile (each matmul
                # stays within one bank) so ONE exp covers two blocks —
                # halves the ACT per-op overhead, which otherwise saturates
                # the scalar engine during the attention phase.
                # groups: list of (j, trim, base) packed into one tile
                nfull = 4 * qp
                groups = [
                    [(2 * p, 0, 0), (2 * p + 1, 0, 512)]
                    for p in range(nfull // 2)
                ]
                # diagonal band: widths 512,384 share a tile; 256,128 share
                groups.append([(nfull, 0, 0), (nfull + 1, 128, 512)])
                groups.append([(nfull + 2, 256, 0), (nfull + 3, 384, 256)])
                pts = [None] * (nfull + 4)
                for grp in groups:
                    width = max(base + 512 - trim for (_, trim, base) in grp)
                    ps = pmm.tile([128, 1024], f32, tag="mm", name="ps")
                    pt = ptp.tile([128, 1024], bf16, tag="pt", name="pt")
                    for (j, trim, base) in grp:
                        w = 512 - trim
                        q0 = s * T + qp * 512 + trim
                        nc.tensor.matmul(
                            ps[:, base : base + w],
                            lhsT=kT_sb[:, j * 128 : (j + 1) * 128],
                            rhs=qT_sb[:, q0 : q0 + w],
                            start=True,
                            stop=True,
                        )
                    nc.scalar.activation(
                        pt[:, 0:width], ps[:, 0:width],
                        mybir.ActivationFunctionType.Exp, scale=SCALE,
                    )
                    for (j, trim, base) in grp:
                        if j - nfull >= 0:
                            # only the first 128-col band straddles the diagonal
                            nc.vector.tensor_mul(
                                pt[:, base : base + 128], pt[:, base : base + 128], mask
                            )
                        pts[j] = (pt, trim, base)
                return pts

            def emit_av(qp, s, pts):
                norms = []
                for u in range(4):
                    jmax = 4 * qp + u
                    # short chains early on: rotate over 4 banks (oext+tr) so
                    # the chain head never waits on DVE normalization drain
                    if qp < 2 and u % 2 == 1:
                        oe = ptr.tile([128, HD + 1], f32, tag="tr", name="oe")
                    else:
                        oe = pox.tile([128, HD + 1], f32, tag="oext", name="oe")
                    for j in range(jmax + 1):
                        pt, trim, base = pts[j]
                        c0 = base + u * 128 - trim
                        nc.tensor.matmul(
                            oe,
                            lhsT=pt[:, c0 : c0 + 128],
                            rhs=vext_sb[:, j * (HD + 1) : (j + 1) * (HD + 1)],
                            start=(j == 0),
                            stop=(j == jmax),
                        )
                    rc = smp.tile([128, 1], f32, tag="rc", name="rc")
                    nc.vector.reciprocal(rc, oe[:, HD : HD + 1])
                    on = onp.tile([128, 128], bf16, tag="on", name="on")
                    nc.vector.tensor_scalar_mul(on, oe[:, 0:HD], rc)
                    norms.append((on, s, qp * 512 + u * 128))
                return norms

            def emit_epilogue(qp):
                # output projection for this q-pass's 4 row tiles, staged DMA
                # per jp so the output drains early. PSUM rotates over FOUR
                # banks (oext x2 + tr x2) and the staging copies alternate
                # DVE/ACT so the matmul chains never wait on bank evacuation.
                for u in range(4):
                    tt = qp * 4 + u
                    for jp in range(4):
                        pool = pox if jp % 2 == 0 else ptr
                        tag = "oext" if jp % 2 == 0 else "tr"
                        ps = pool.tile([128, 512], f32, tag=tag, name="ps")
                        for s in range(GS):
                            nc.tensor.matmul(
                                ps,
                                lhsT=ohT_sb[:, s * T + tt * 128 : s * T + (tt + 1) * 128],
                                rhs=wp_sb[:, s * EMB + jp * 512 : s * EMB + (jp + 1) * 512],
                                start=(s == 0),
                                stop=(s == GS - 1),
                            )
                        ot = osp.tile([128, 512], bf16, tag="ostage", name="ot")
                        # staging alternates DVE/ACT early on, but during the
                        # qp>=2 stretch ACT runs ~90% busy on exp — keep its
                        # FIFO clear and stage on DVE only there; DMA issues
                        # ride the otherwise-idle sync queue
                        if qp >= 2 or (u * 4 + jp) % 2 == 0:
                            nc.vector.tensor_copy(ot, ps)
                        else:
                            nc.scalar.copy(ot, ps)
                        nc.sync.dma_start(
                            out=out_d[tt, :, jp * 512 : (jp + 1) * 512], in_=ot
                        )


            last_norms = []

            def emit_transposes(norms):
                for on, s, tq in norms:
                    tps = ptr.tile([128, 128], bf16, tag="tr", name="tps")
                    nc.tensor.transpose(tps, on, ident)
                    nc.vector.tensor_copy(
                        ohT_sb[:, s * T + tq : s * T + tq + 128], tps
                    )

            def advance(pending):
                # AV for the pending iteration, then the (lag-1) transposes of
                # the previous one; at a q-pass boundary flush and project
                nonlocal last_norms
                qp, s, pts = pending
                norms = emit_av(qp, s, pts)
                emit_transposes(last_norms)
                last_norms = norms
                if s == GS - 1:
                    emit_transposes(last_norms)
                    last_norms = []
                    emit_epilogue(qp)

            window = []

            def push(qp, s, maxw):
                pts = emit_scores(qp, s)
                window.append((qp, s, pts))
                if len(window) > maxw:
                    advance(window.pop(0))

            # Q projection per head, in half-head groups that ping-pong the
            # two 2-bank "mm" slots (copy of group k overlaps chains of group
            # k+1). The qp=0 attention iteration for head s rides along right
            # after its qT is staged, so attention's shallow-pipeline start
            # hides inside dense Q-projection matmul work.
            for s in range(GS):
                for h, tps in enumerate(((0, 1), (2, 3))):
                    pg = pmm.tile([128, 1024], f32, tag="mm", name="pg")
                    for c in range(NE):
                        for ti, tp in enumerate(tps):
                            nc.tensor.matmul(
                                pg[:, ti * 512 : (ti + 1) * 512],
                                lhsT=wq_sb[
                                    :, c * GS * HD + s * HD : c * GS * HD + (s + 1) * HD
                                ],
                                rhs=xT_sb[:, c * T + tp * 512 : c * T + (tp + 1) * 512],
                                start=(c == 0),
                                stop=(c == NE - 1),
                            )
                    dst = qT_sb[:, s * T + tps[0] * 512 : s * T + (tps[1] + 1) * 512]
                    if (2 * s + h) % 2 == 0:
                        nc.vector.tensor_copy(dst, pg)
                    else:
                        nc.scalar.copy(dst, pg)
                push(0, s, 3)
            for qp in range(1, NQP):
                for s in range(GS):
                    # deeper pipeline early on: qp=1 AV chains are short, so
                    # PE needs more queued scores to ride out ACT exp latency
                    push(qp, s, 3 if qp < 2 else 2)
            for w in window:
                advance(w)

    nc.finalize()
    return nc


def _get_program():
    global _PROGRAM
    if _PROGRAM is None:
        _PROGRAM = _build_program()
    return _PROGRAM


def _pack(a, nchunk):
    """[nchunk*128, F] -> [128, nchunk*F] so it lands in SBUF layout with one
    contiguous DMA: out[p, c*F + f] = a[c*128 + p, f]."""
    n, f = a.shape
    assert n == nchunk * 128
    return np.ascontiguousarray(
        a.reshape(nchunk, 128, f).transpose(1, 0, 2).reshape(128, nchunk * f)
    )


def _make_in_maps(x, Wq, Wk, Wv, Wp):
    # convert to numpy up front: slicing a jax array would trace/compile
    # a jax op per slice instead of cheap host-side numpy views
    x, Wq, Wk, Wv, Wp = (np.asarray(a) for a in (x, Wq, Wk, Wv, Wp))
    in_maps = []
    xTs = [_pack(x[b].T.astype(_BF16), NE) for b in range(2)]
    for c in range(8):
        b, g = c // 4, c % 4
        sl = slice(g * GS * HD, (g + 1) * GS * HD)
        kv = slice(g * GS * HD, g * GS * HD + HD)
        in_maps.append(
            {
                "xTp": xTs[b],
                "wqp": _pack(Wq[sl, :].T.astype(_BF16), NE),
                "wkp": _pack(Wk[kv, :].T.astype(_BF16), NE),
                "wvp": _pack(Wv[kv, :].T.astype(_BF16), NE),
                "wpp": _pack(Wp[:, sl].T.astype(_BF16), GS),
            }
        )
    return in_maps


def run(x, Wq, Wk, Wv, Wp, bp, trace=False, **trace_kwargs):
    from concourse.bass_utils import run_bass_kernel_spmd

    nc = _get_program()
    in_maps = _make_in_maps(x, Wq, Wk, Wv, Wp)
    res = run_bass_kernel_spmd(
        nc, in_maps, core_ids=list(range(8)), trace=trace, **trace_kwargs
    )
    bp = np.asarray(bp, dtype=np.float32)
    y = np.empty((2, T, EMB), dtype=np.float32)
    for b in range(2):
        acc = res.results[4 * b]["partial"].astype(np.float32)
        for g in range(1, 4):
            acc += res.results[4 * b + g]["partial"].astype(np.float32)
        y[b] = acc + bp
    return y, res


def kernel(x, Wq, Wk, Wv, Wp, bp):
    y, _ = run(x, Wq, Wk, Wv, Wp, bp, trace=False)
    return y

